# revision 1
# baseline (speedup 1.0000x reference)
"""Trainium2 Bass kernel for a 6-layer GAT GNN (nn_GAT_GNN_35579509080109).

Strategy (8 NeuronCores, node partition):
  - Nodes are degree-balanced into 160 blocks of 128 slots (125 real nodes
    each); each device owns 20 blocks (2560 padded node slots).
  - Per layer, each device computes hw = h @ [W | W a_src | W a_dst] (+ folded
    bias) for its own nodes, packs a 768B row table
    [hw(256) bf16 | 1.0 | pad | e_src f32 | e_dst f32 | pad..384], and
    AllGathers the table. The front MLP (W1@W2) is folded into layer 0's
    weights, so layer 0 consumes xT directly.
  - Edges are partitioned by destination owner, sorted into dst blocks, and
    processed in chunks of 128 edges: hw[src]+e_src via a batched dma_gather
    (768B rows, by global src row). Per-edge index streams (srcw, slotf) are
    layer-invariant and stay resident in SBUF (loaded once).
  - e_dst per edge comes from a host-precomputed static one-hot transpose
    ptT[slot, edge] (fp8, SBUF-resident, 5.2MB): eb = ptT^T @ e_dst_col per
    chunk on the tensor engine (no aux dma_gather). These broadcasts depend
    only on local hw, so they overlap the AllGather.
  - Per-edge softmax numerators ee = exp(leaky_relu(e_src+e_dst)) computed as
    max(exp(x), exp(0.2 x)) on the scalar engine in [128, GC] batches.
  - Scatter-add + denominators on the tensor engine: one-hot(dst slot) * ee
    (lhsT, bf16) x [hw | 1] (rhs) accumulates [128 dst, 257] in PSUM per
    block; out = psum[:, :256] / psum[:, 256].
  - Layer bias folds into the next layer's matmul (extra K=1 row); the final
    concat(h, h) @ W3 collapses to relu(h) @ (W3_top + W3_bot).

Notes from tuning (vs the earlier baseline: sim 2.64ms -> 2.38ms; paired
real-HW A/B deltas -1.1ms and -2.6ms/call across two sessions; rel err
1.158e-3 vs the 2e-2 gate):
  - The 256B-row aux dma_gather for e_dst was ~0.44ms of DMA busy across the
    run; replaced by the static fp8 ptT matmuls (PE has headroom).
  - Real HW punishes small-DMA dispatch overhead 4-10x beyond the cost
    model; that is why the aux-gather removal, persistent index streams,
    and the batched (4-blocks-per-DMA) output writes out-performed their
    sim deltas.
  - fp8 (e4m3) for the 256-value hw payload works mechanically (512B rows,
    with per-layer scaling to dodge underflow: activations decay ~5x/layer
    and unscaled values silently flush to zero) but lands at rel err 1.7e-2
    vs the 2e-2 gate - too close; kept bf16.
  - Do NOT exceed 1024 indices per dma_gather: a 2048-idx build compiles
    and simulates cleanly but hangs the device at first execution.
  - remote_dma_broadcast + tc.Switch would replace the 162us/layer (real)
    AllGather with overlapped p2p writes, but the Tile scheduler's no_exec
    cost model cannot deliver remote sem updates ("known gap"), so any
    receiver-side wait deadlocks schedule_block. Dead end in this build.
  - Also closed: strided/sliced collective APs (NEFF compile reject),
    split/hierarchical/AllReduce/AllToAll exchanges (size-cost curve),
    layer-fusion code motion (+13us PE/PSUM contention), deeper buffer
    rotations (SBUF/PSUM at capacity). Remaining floors: the AllGather and
    the 768B/edge gather stride (256B-multiple HW minimum for hw+e_src).
"""
import os
import sys
import numpy as np

for _p in ("/opt/trn_rl_repo", "/root/.axon_site/_ro/trn_rl_repo"):
    if os.path.isdir(_p) and _p not in sys.path:
        sys.path.append(_p)

# ---------------- problem constants ----------------
N = 20000
E = 320000
D = 256
NEG = 0.2
NDEV = 8

GC = 8    # chunks per gather group (1024 edges / dma_gather call; HW limit ~1024 idxs)
RW = 384  # table row width in bf16 (768 bytes)


class Cfg:
    def __init__(self, n, e, bpd):
        self.n, self.e, self.bpd = n, e, bpd
        self.npd = bpd * 128
        self.nblk = NDEV * bpd

FULL = Cfg(N, E, 20)


def _wrap16(flat):
    """dma_gather index layout: idx i at [i%16, i//16], replicated to 128 rows."""
    ni = flat.shape[0]
    w = np.ascontiguousarray(flat.reshape(ni // 16, 16).T).astype(np.int16)
    return np.tile(w, (8, 1))


# ---------------- host preprocessing ----------------
def prep(inputs, cfg):
    x = np.ascontiguousarray(np.asarray(inputs["x"], np.float32))
    ei = np.asarray(inputs["edge_index"]).astype(np.int64)
    W1 = np.asarray(inputs["W1"], np.float32)
    W2 = np.asarray(inputs["W2"], np.float32)
    Ws = np.asarray(inputs["Ws"], np.float32)
    a_src = np.asarray(inputs["a_src"], np.float32)
    a_dst = np.asarray(inputs["a_dst"], np.float32)
    bias = np.asarray(inputs["bias"], np.float32)
    W3 = np.asarray(inputs["W3"], np.float32)
    src, dst = ei[0], ei[1]
    n, bpd, npd, nblk = cfg.n, cfg.bpd, cfg.npd, cfg.nblk

    # --- degree-balanced node -> (dev, blk, slot) assignment (snake) ---
    deg = np.bincount(dst, minlength=n)
    order = np.argsort(-deg, kind="stable")
    r = np.arange(n)
    stripe = r // nblk
    posin = r % nblk
    blk_glob = np.where(stripe % 2 == 0, posin, nblk - 1 - posin)
    slot = stripe
    assert slot.max() < 128
    pos = np.empty(n, np.int64)
    pos[order] = (blk_glob // bpd) * npd + (blk_glob % bpd) * 128 + slot

    # --- edge grouping by dst block ---
    dstp = pos[dst]
    bid = dstp // npd * bpd + (dstp % npd) // 128  # global block id
    sidx = np.argsort(bid, kind="stable")
    counts = np.bincount(bid, minlength=nblk)
    cpb = int(np.ceil(counts.max() / 128))
    nchunk_raw = bpd * cpb
    nchunk = ((nchunk_raw + GC - 1) // GC) * GC
    starts = np.zeros(nblk + 1, np.int64)
    starts[1:] = np.cumsum(counts)
    rank = np.arange(cfg.e) - starts[bid[sidx]]

    sdev = (dstp // npd)[sidx]
    sblk = ((dstp % npd) // 128)[sidx]
    sslot = (dstp % 128)[sidx]
    ssrc = pos[src][sidx]
    kk = sblk * cpb + rank // 128
    pp = rank % 128

    SRC = np.zeros((NDEV, 128, nchunk), np.int32)       # global table row of src
    SLOT = np.full((NDEV, 128, nchunk), 255.0, np.float32)
    SRC[sdev, pp, kk] = ssrc
    SLOT[sdev, pp, kk] = sslot

    # wrapped int16 index arrays for dma_gather, per group of GC chunks
    ng = nchunk // GC
    wcols = GC * 128 // 16
    srcw = np.zeros((NDEV, 128, wcols * ng), np.int16)
    for dv in range(NDEV):
        for g in range(ng):
            # edge i in group = c*128 + p, c in [0,GC)
            flat_s = SRC[dv][:, g * GC:(g + 1) * GC].T.reshape(-1)  # [GC*128] c-major
            srcw[dv][:, g * wcols:(g + 1) * wcols] = _wrap16(flat_s)

    # --- x permuted / padded / transposed ---
    xp = np.zeros((NDEV, npd, D), np.float32)
    xp[pos // npd, pos % npd] = x
    xpT = np.ascontiguousarray(xp.transpose(0, 2, 1))

    # --- weights ---
    W12 = np.ascontiguousarray(W1 @ W2)
    wfull = np.zeros((6, 257, 258), np.float32)
    for l in range(6):
        wext = np.concatenate(
            [Ws[l], (Ws[l] @ a_src[l])[:, None], (Ws[l] @ a_dst[l])[:, None]], axis=1
        )
        # layer 0 consumes x directly: fold the front MLP (W1 @ W2) in
        wfull[l, :256] = (W12 @ wext) if l == 0 else wext
        if l >= 1:
            wfull[l, 256] = bias[l - 1] @ wext
    W3s = np.ascontiguousarray(W3[:256] + W3[256:])
    bias6 = np.tile(bias[5][None, :], (128, 1)).astype(np.float32)
    iotaf = np.tile(np.arange(128, dtype=np.float32)[None, :], (128, 1))
    # static one-hot transpose per chunk: ptT[slot, k*128+e] = (slot(e,k) == slot)
    from concourse import mybir as _mb
    f8 = _mb.dt.np(_mb.dt.float8e4)
    PT8 = np.zeros((NDEV, 128, nchunk * 128), f8)
    for dv in range(NDEV):
        S = SLOT[dv].astype(np.int32)          # [128 e, nchunk k]
        e_i, k_i = np.nonzero(S < 128)
        PT8[dv][S[e_i, k_i], k_i * 128 + e_i] = 1.0

    in_maps = []
    for dv in range(NDEV):
        in_maps.append(
            {
                "xT": np.ascontiguousarray(xpT[dv]),
                "srcw": np.ascontiguousarray(srcw[dv]),
                "slotf": np.ascontiguousarray(SLOT[dv]),
                "wfull": wfull,
                "w3s": W3s,
                "bias6": bias6,
                "iotaf": iotaf,
                "pt8": PT8[dv],
            }
        )
    return in_maps, pos, cpb, nchunk


# ---------------- bass program ----------------
def build(cfg, cpb, nchunk):
    import concourse.bass as bass
    import concourse.bacc as bacc
    import concourse.tile as tile
    from concourse import mybir
    from concourse.masks import make_identity

    f32 = mybir.dt.float32
    f32r = mybir.dt.float32r
    bf16 = mybir.dt.bfloat16
    i16 = mybir.dt.int16
    AF = mybir.ActivationFunctionType
    OP = mybir.AluOpType
    npd, bpd = cfg.npd, cfg.bpd
    ng = nchunk // GC

    nc = bacc.Bacc(
        "TRN2",
        target_bir_lowering=False,
        debug=False,
        enable_asserts=False,
        num_devices=NDEV,
    )
    xT = nc.dram_tensor("xT", [256, npd], f32, kind="ExternalInput").ap()
    wcols = GC * 128 // 16
    srcw = nc.dram_tensor("srcw", [128, wcols * ng], i16, kind="ExternalInput").ap()
    slotf = nc.dram_tensor("slotf", [128, nchunk], f32, kind="ExternalInput").ap()
    pt8 = nc.dram_tensor("pt8", [128, nchunk * 128], mybir.dt.float8e4,
                         kind="ExternalInput").ap()
    wfull = nc.dram_tensor("wfull", [6, 257, 258], f32, kind="ExternalInput").ap()
    w3s = nc.dram_tensor("w3s", [256, 256], f32, kind="ExternalInput").ap()
    bias6 = nc.dram_tensor("bias6", [128, 256], f32, kind="ExternalInput").ap()
    iotaf = nc.dram_tensor("iotaf", [128, 128], f32, kind="ExternalInput").ap()
    out = nc.dram_tensor("out", [npd, 256], f32, kind="ExternalOutput").ap()

    with tile.TileContext(nc) as tc:
        with (
            tc.tile_pool(name="cp", bufs=1) as cp,
            tc.tile_pool(name="sb", bufs=2) as sb,
            tc.tile_pool(name="gp", bufs=5) as gp,
            tc.tile_pool(name="psA", bufs=3, space="PSUM") as psA,
            tc.tile_pool(name="psB", bufs=2, space="PSUM") as psB,
            tc.tile_pool(name="dp", bufs=1, space="DRAM") as dp,
        ):
            # ---- constants ----
            ident = cp.tile([128, 128], f32)
            make_identity(nc, ident[:])
            iota_f = cp.tile([128, 128], f32)
            nc.sync.dma_start(iota_f[:], iotaf)
            iota_b = cp.tile([128, 128], bf16)
            nc.vector.tensor_copy(iota_b[:], iota_f[:])
            # layer-invariant edge indexing data, resident in SBUF
            pt8_sb = cp.tile([128, nchunk * 128], mybir.dt.float8e4)
            nc.sync.dma_start(pt8_sb[:], pt8)
            srcw_all = cp.tile([128, wcols * ng], i16)
            nc.sync.dma_start(srcw_all[:], srcw)
            slot_all = cp.tile([128, nchunk], f32)
            nc.sync.dma_start(slot_all[:], slotf)
            ones1f = cp.tile([1, 128], f32)
            nc.gpsimd.memset(ones1f[:], 1.0)
            ones1 = cp.tile([1, 128], f32r)
            nc.vector.tensor_copy(ones1[:], ones1f[:])
            b6sb = cp.tile([128, 256], f32)
            nc.sync.dma_start(b6sb[:], bias6)
            # stage fp32 loads through a small buffer, DVE-cast to f32r
            wextsb = cp.tile([128, 6, 2, 258], f32r)
            bwsb = cp.tile([1, 6, 258], f32r)
            w3ssb = cp.tile([128, 2, 256], f32r)

            def _stage(dst_ap, src_ap, shape):
                st = sb.tile(list(shape), f32, tag="wstage", bufs=2, name="wstage")
                nc.sync.dma_start(st[:], src_ap)
                nc.vector.tensor_copy(dst_ap, st[:])

            for l in range(6):
                _stage(
                    wextsb[:, l],
                    wfull[l, 0:256, :].rearrange("(a p) c -> p a c", p=128),
                    [128, 2, 258],
                )
            _stage(bwsb[:], wfull[:, 256:257, :].rearrange("l o c -> o l c"),
                   [1, 6, 258])
            _stage(w3ssb[:], w3s.rearrange("(a p) m -> p a m", p=128), [128, 2, 256])

            # ---- DRAM comm buffers (per layer: Shared tensors allow one writer) ----
            tbl_owns = [dp.tile([npd, RW], bf16, name=f"tbl_own{i}") for i in range(6)]
            tbl_fulls = [
                dp.tile([NDEV * npd, RW], bf16, addr_space="Shared", name=f"tbl_full{i}")
                for i in range(6)
            ]

            # ---- layer-0 hT is just xT (W1@W2 folded into wfull[0]) ----
            hT = sb.tile([128, 2, npd], f32r, tag="hT")
            xr = xT.rearrange("(a p) n -> p a n", p=128)
            xstep = min(512, npd)
            for n0 in range(0, npd, xstep):
                _stage(hT[:, :, n0:n0 + xstep], xr[:, :, n0:n0 + xstep],
                       [128, 2, xstep])

            for l in range(6):
                tbl_own, tbl_full = tbl_owns[l], tbl_fulls[l]
                # ---- own-node hw + table build ----
                tbl_sb = sb.tile([128, bpd, RW], bf16, tag="tbl")
                tblf = tbl_sb[:].bitcast(f32)  # [128, bpd, RW//2]
                nc.gpsimd.memset(tbl_sb[:, :, 256:RW], 1.0)
                edst_all = sb.tile([128, bpd], bf16, tag="edst")
                for b in range(bpd):
                    pshw = psB.tile([128, 258], f32, tag="pshw")
                    for ki in range(2):
                        nc.tensor.matmul(
                            pshw[:],
                            lhsT=hT[:, ki, b * 128 : (b + 1) * 128],
                            rhs=wextsb[:, l, ki, :],
                            start=(ki == 0),
                            stop=False,
                        )
                    nc.tensor.matmul(
                        pshw[:],
                        lhsT=ones1[:],
                        rhs=bwsb[:, l, :],
                        start=False,
                        stop=True,
                    )
                    nc.vector.tensor_copy(tbl_sb[:, b, 0:256], pshw[:, 0:256])
                    nc.vector.tensor_copy(tblf[:, b, 129:130], pshw[:, 256:257])
                    nc.vector.tensor_copy(edst_all[:, b : b + 1], pshw[:, 257:258])
                nc.sync.dma_start(
                    tbl_own[:].rearrange("(b p) c -> p b c", p=128), tbl_sb[:]
                )
                nc.gpsimd.collective_compute(
                    "AllGather",
                    mybir.AluOpType.bypass,
                    replica_groups=[list(range(NDEV))],
                    ins=[tbl_own[:]],
                    outs=[tbl_full[:]],
                )

                # ---- e_dst per edge for ALL groups (overlaps the AllGather:
                # depends only on edst_all + the static one-hot transpose) ----
                EB_all = sb.tile([128, nchunk], f32, tag="eball")
                for g in range(ng):
                    pseb = psB.tile([128, GC], f32, tag="pseb", bufs=2)
                    for c in range(GC):
                        k = g * GC + c
                        b = k // cpb
                        nc.tensor.matmul(
                            pseb[:, c : c + 1],
                            lhsT=pt8_sb[:, k * 128:(k + 1) * 128],
                            rhs=edst_all[:, b : b + 1],
                            start=True,
                            stop=True,
                        )
                    nc.vector.tensor_copy(EB_all[:, g * GC:(g + 1) * GC], pseb[:])

                # ---- gather / scatter ----
                hT_next = sb.tile([128, 2, npd], f32r, tag="hT")
                pss = None
                for g in range(ng):
                    # --- group prologue: src gather + e_dst broadcast + ee ---
                    G = gp.tile([128, GC, RW], bf16, tag="G")
                    nc.gpsimd.dma_gather(
                        out_ap=G[:], in_ap=tbl_full[:],
                        idxs_ap=srcw_all[:, g * wcols:(g + 1) * wcols],
                        num_idxs=GC * 128, num_idxs_reg=GC * 128,
                        elem_size=RW,
                    )
                    Gf = G[:].bitcast(f32)    # [128, GC, 192]
                    X = gp.tile([128, 2 * GC], f32, tag="X")
                    nc.vector.tensor_tensor(
                        X[:, 0:GC], Gf[:, :, 129],
                        EB_all[:, g * GC:(g + 1) * GC], op=OP.add
                    )
                    nc.vector.tensor_scalar(
                        out=X[:, GC:2 * GC], in0=X[:, 0:GC], scalar1=NEG,
                        scalar2=None, op0=OP.mult,
                    )
                    E1 = gp.tile([128, 2 * GC], f32, tag="E1")
                    nc.scalar.activation(E1[:], X[:], AF.Exp)
                    EE = gp.tile([128, GC], f32, tag="EE")
                    nc.vector.tensor_tensor(
                        EE[:], E1[:, 0:GC], E1[:, GC:2 * GC], op=OP.max
                    )
                    # --- scatter pass ---
                    for c in range(GC):
                        k = g * GC + c
                        b, cc = divmod(k, cpb)
                        if cc == 0:
                            pss = psA.tile([128, 257], f32, tag="pss")
                        lt = gp.tile([128, 128], bf16, tag="lt")
                        nc.vector.tensor_scalar(
                            out=lt[:],
                            in0=iota_b[:],
                            scalar1=slot_all[:, k : k + 1],
                            scalar2=EE[:, c : c + 1],
                            op0=OP.is_equal,
                            op1=OP.mult,
                        )
                        nc.tensor.matmul(
                            pss[:],
                            lhsT=lt[:],
                            rhs=G[:, c, 0:257],
                            start=(cc == 0),
                            stop=(cc == cpb - 1),
                        )
                        if cc != cpb - 1:
                            continue
                        # ---- block epilogue: normalize ----
                        den = sb.tile([128, 1], f32, tag="den")
                        nc.vector.tensor_scalar(
                            out=den[:], in0=pss[:, 256:257], scalar1=1e-30,
                            scalar2=None, op0=OP.add,
                        )
                        rec = sb.tile([128, 1], f32, tag="rec")
                        nc.vector.reciprocal(rec[:], den[:])
                        onrm = sb.tile([128, 256], f32, tag="onrm")
                        nc.scalar.activation(
                            onrm[:], pss[:, 0:256], AF.Copy, scale=rec[:]
                        )
                        if l == 5:
                            ob = sb.tile([128, 256], f32, tag="ob")
                            nc.vector.tensor_tensor(ob[:], onrm[:], b6sb[:], op=OP.add)
                            orl = sb.tile([128, 256], f32, tag="orl")
                            nc.vector.tensor_scalar(
                                out=orl[:], in0=ob[:], scalar1=0.0, scalar2=None,
                                op0=OP.max,
                            )
                            srct_t = orl
                        else:
                            srct_t = onrm
                        pst = psB.tile([128, 256], f32, tag="pst", bufs=1)
                        for hh in range(2):
                            nc.tensor.transpose(
                                out=pst[:, hh * 128 : (hh + 1) * 128],
                                in_=srct_t[:, hh * 128 : (hh + 1) * 128],
                                identity=ident[:],
                            )
                            nc.vector.tensor_copy(
                                hT_next[:, hh, b * 128 : (b + 1) * 128],
                                pst[:, hh * 128 : (hh + 1) * 128],
                            )
                hT = hT_next

            # ---- final: relu(h6+bias) @ (W3_top + W3_bot) ----
            # batch 4 node blocks per output DMA: fewer HWDGE dispatches
            OB = 4
            for b0 in range(0, bpd, OB):
                nb = min(OB, bpd - b0)
                oo = sb.tile([128, OB, 256], f32, tag="oo")
                for bi in range(nb):
                    b = b0 + bi
                    psf = psB.tile([128, 256], f32, tag="pshw")
                    for ki in range(2):
                        nc.tensor.matmul(
                            psf[:],
                            lhsT=hT[:, ki, b * 128 : (b + 1) * 128],
                            rhs=w3ssb[:, ki, :],
                            start=(ki == 0),
                            stop=(ki == 1),
                        )
                    nc.vector.tensor_copy(oo[:, bi, :], psf[:])
                nc.sync.dma_start(
                    out[b0 * 128 : (b0 + nb) * 128, :].rearrange(
                        "(b p) c -> p b c", p=128
                    ),
                    oo[:, 0:nb, :],
                )

    nc.compile()
    return nc


# ---------------- entry point ----------------
def kernel(**inputs):
    cfg = FULL
    in_maps, pos, cpb, nchunk = prep(inputs, cfg)
    nc = build(cfg, cpb, nchunk)
    from concourse import bass_utils

    res = bass_utils.run_bass_kernel_spmd(nc, in_maps, core_ids=list(range(NDEV)))
    outs = [res.results[dv]["out"] for dv in range(NDEV)]
    full = np.zeros((cfg.n, 256), np.float32)
    full[:] = np.stack(outs).reshape(NDEV * cfg.npd, 256)[pos]
    return full



# revision 18
# speedup vs baseline: 1.3597x; 1.3597x over previous
"""Trainium2 Bass kernel for a 6-layer GAT GNN (nn_GAT_GNN_35579509080109).

Strategy (8 NeuronCores, node partition):
  - Nodes are degree-balanced into 160 blocks of 128 slots (125 real nodes
    each); each device owns 20 blocks (2560 padded node slots).
  - Per layer, each device computes hw = h @ [W | W a_src | W a_dst] (+ folded
    bias) for its own nodes, packs a 768B row table
    [hw(256) bf16 | 1.0 | pad | e_src f32 | e_dst f32 | pad..384], and
    AllGathers the table. The front MLP (W1@W2) is folded into layer 0's
    weights, so layer 0 consumes xT directly.
  - Edges are partitioned by destination owner, sorted into dst blocks, and
    processed in chunks of 128 edges: hw[src]+e_src via a batched dma_gather
    (768B rows, by global src row). Per-edge index streams (srcw, slotf) are
    layer-invariant and stay resident in SBUF (loaded once).
  - e_dst per edge comes from a host-precomputed static one-hot transpose
    ptT[slot, edge] (fp8, SBUF-resident, 5.2MB): eb = ptT^T @ e_dst_col per
    chunk on the tensor engine (no aux dma_gather). These broadcasts depend
    only on local hw, so they overlap the AllGather.
  - Per-edge softmax numerators ee = exp(leaky_relu(e_src+e_dst)) computed as
    max(exp(x), exp(0.2 x)) on the scalar engine in [128, GC] batches.
  - Scatter-add + denominators on the tensor engine: one-hot(dst slot) * ee
    (lhsT, bf16) x [hw | 1] (rhs) accumulates [128 dst, 257] in PSUM per
    block; out = psum[:, :256] / psum[:, 256].
  - Layer bias folds into the next layer's matmul (extra K=1 row); the final
    concat(h, h) @ W3 collapses to relu(h) @ (W3_top + W3_bot).

Notes from tuning (vs the earlier baseline: sim 2.64ms -> 2.38ms; paired
real-HW A/B deltas -1.1ms and -2.6ms/call across two sessions; rel err
1.158e-3 vs the 2e-2 gate):
  - The 256B-row aux dma_gather for e_dst was ~0.44ms of DMA busy across the
    run; replaced by the static fp8 ptT matmuls (PE has headroom).
  - Real HW punishes small-DMA dispatch overhead 4-10x beyond the cost
    model; that is why the aux-gather removal, persistent index streams,
    and the batched (4-blocks-per-DMA) output writes out-performed their
    sim deltas.
  - fp8 (e4m3) for the 256-value hw payload works mechanically (512B rows,
    with per-layer scaling to dodge underflow: activations decay ~5x/layer
    and unscaled values silently flush to zero) but lands at rel err 1.7e-2
    vs the 2e-2 gate - too close; kept bf16.
  - Do NOT exceed 1024 indices per dma_gather: a 2048-idx build compiles
    and simulates cleanly but hangs the device at first execution.
  - remote_dma_broadcast + tc.Switch would replace the 162us/layer (real)
    AllGather with overlapped p2p writes, but the Tile scheduler's no_exec
    cost model cannot deliver remote sem updates ("known gap"), so any
    receiver-side wait deadlocks schedule_block. Dead end in this build.
  - Also closed: strided/sliced collective APs (NEFF compile reject),
    split/hierarchical/AllReduce/AllToAll exchanges (size-cost curve),
    layer-fusion code motion (+13us PE/PSUM contention), deeper buffer
    rotations (SBUF/PSUM at capacity). Remaining floors: the AllGather and
    the 768B/edge gather stride (256B-multiple HW minimum for hw+e_src).
"""
import os
import sys
import numpy as np

for _p in ("/opt/trn_rl_repo", "/root/.axon_site/_ro/trn_rl_repo"):
    if os.path.isdir(_p) and _p not in sys.path:
        sys.path.append(_p)

# ---------------- problem constants ----------------
N = 20000
E = 320000
D = 256
NEG = 0.2
NDEV = 8

GC = 8    # chunks per gather group (1024 edges / dma_gather call; HW limit ~1024 idxs)
RW = 256  # table row width in bf16 (512 bytes): rotated hw' only
# timing-probe knobs (correctness only guaranteed for defaults)
AG_MODE = os.environ.get("KAG", "full")
SKIP = os.environ.get("KSKIP", "")


class Cfg:
    def __init__(self, n, e, bpd):
        self.n, self.e, self.bpd = n, e, bpd
        self.npd = bpd * 128
        self.nblk = NDEV * bpd

FULL = Cfg(N, E, 20)


def _wrap16(flat):
    """dma_gather index layout: idx i at [i%16, i//16], replicated to 128 rows."""
    ni = flat.shape[0]
    w = np.ascontiguousarray(flat.reshape(ni // 16, 16).T).astype(np.int16)
    return np.tile(w, (8, 1))


# ---------------- host preprocessing ----------------
def prep(inputs, cfg):
    x = np.ascontiguousarray(np.asarray(inputs["x"], np.float32))
    ei = np.asarray(inputs["edge_index"]).astype(np.int64)
    W1 = np.asarray(inputs["W1"], np.float32)
    W2 = np.asarray(inputs["W2"], np.float32)
    Ws = np.asarray(inputs["Ws"], np.float32)
    a_src = np.asarray(inputs["a_src"], np.float32)
    a_dst = np.asarray(inputs["a_dst"], np.float32)
    bias = np.asarray(inputs["bias"], np.float32)
    W3 = np.asarray(inputs["W3"], np.float32)
    src, dst = ei[0], ei[1]
    n, bpd, npd, nblk = cfg.n, cfg.bpd, cfg.npd, cfg.nblk

    # --- degree-balanced node -> (dev, blk, slot) assignment (snake) ---
    deg = np.bincount(dst, minlength=n)
    order = np.argsort(-deg, kind="stable")
    r = np.arange(n)
    stripe = r // nblk
    posin = r % nblk
    blk_glob = np.where(stripe % 2 == 0, posin, nblk - 1 - posin)
    slot = stripe
    assert slot.max() < 128
    pos = np.empty(n, np.int64)
    pos[order] = (blk_glob // bpd) * npd + (blk_glob % bpd) * 128 + slot

    # --- edge grouping by dst block ---
    dstp = pos[dst]
    bid = dstp // npd * bpd + (dstp % npd) // 128  # global block id
    sidx = np.argsort(bid, kind="stable")
    counts = np.bincount(bid, minlength=nblk)
    cpb = int(np.ceil(counts.max() / 128))
    nchunk_raw = bpd * cpb
    nchunk = ((nchunk_raw + GC - 1) // GC) * GC
    starts = np.zeros(nblk + 1, np.int64)
    starts[1:] = np.cumsum(counts)
    rank = np.arange(cfg.e) - starts[bid[sidx]]

    sdev = (dstp // npd)[sidx]
    sblk = ((dstp % npd) // 128)[sidx]
    sslot = (dstp % 128)[sidx]
    ssrc = pos[src][sidx]
    kk = sblk * cpb + rank // 128
    pp = rank % 128

    SRC = np.zeros((NDEV, 128, nchunk), np.int32)       # global table row of src
    SLOT = np.full((NDEV, 128, nchunk), 255.0, np.float32)
    SRC[sdev, pp, kk] = ssrc
    SLOT[sdev, pp, kk] = sslot

    # wrapped int16 index arrays for dma_gather, per group of GC chunks
    ng = nchunk // GC
    wcols = GC * 128 // 16
    srcw = np.zeros((NDEV, 128, wcols * ng), np.int16)
    for dv in range(NDEV):
        for g in range(ng):
            # edge i in group = c*128 + p, c in [0,GC)
            flat_s = SRC[dv][:, g * GC:(g + 1) * GC].T.reshape(-1)  # [GC*128] c-major
            srcw[dv][:, g * wcols:(g + 1) * wcols] = _wrap16(flat_s)

    # --- x permuted / padded / transposed ---
    xp = np.zeros((NDEV, npd, D), np.float32)
    xp[pos // npd, pos % npd] = x
    xpT = np.ascontiguousarray(xp.transpose(0, 2, 1))

    # --- weights: per-layer rotation folds e_src into hw'[:, 0] ---
    # B'_l = H_l @ diag(||a_src_l||, 1, ...) with Householder H_l e1 =
    # a_src_l/||a_src_l||, so (h @ W_l @ B'_l)[:, 0] = h @ W_l @ a_src_l =
    # e_src and the rest is an orthogonal rotation of hw. The table row is
    # then just 256 bf16 (512B): no separate e_src, no ones column. B'^{-1}
    # folds into layer l+1's weights host-side; layer 5's inverse is applied
    # on-device in the final matmul (binv5).
    W12 = np.ascontiguousarray(W1 @ W2)
    wfull = np.zeros((6, 257, 258), np.float32)
    Binv_prev = None
    for l in range(6):
        u = a_src[l].astype(np.float64)
        nu = float(np.linalg.norm(u))
        e1 = np.zeros(256, np.float64)
        e1[0] = 1.0
        v = u / nu - e1
        vv = float(v @ v)
        H = np.eye(256) - (2.0 / vv) * np.outer(v, v) if vv > 1e-12 else np.eye(256)
        Bp = H.copy()
        Bp[:, 0] *= nu
        Binv = H.copy()
        Binv[0, :] /= nu
        wext = np.concatenate(
            [Ws[l].astype(np.float64) @ Bp,
             (Ws[l] @ a_dst[l]).astype(np.float64)[:, None],
             np.zeros((256, 1))], axis=1
        )  # zero pad col: fp32r matmul needs even free width
        # layer 0 consumes x directly: fold the front MLP (W1 @ W2) in
        if l == 0:
            wfull[l, :256] = (W12.astype(np.float64) @ wext).astype(np.float32)
        else:
            wfull[l, :256] = (Binv_prev @ wext).astype(np.float32)
            wfull[l, 256] = (bias[l - 1].astype(np.float64) @ wext).astype(np.float32)
        Binv_prev = Binv
    W3s = np.ascontiguousarray(W3[:256] + W3[256:])
    binv5 = np.ascontiguousarray(
        Binv_prev.reshape(2, 128, 2, 128).transpose(1, 0, 2, 3)
    ).astype(np.float32)
    b5c = np.ascontiguousarray(bias[5].reshape(2, 128).T).astype(np.float32)
    iotaf = np.tile(np.arange(128, dtype=np.float32)[None, :], (128, 1))
    # static one-hot transpose per chunk: ptT[slot, k*128+e] = (slot(e,k) == slot)
    from concourse import mybir as _mb
    f8 = _mb.dt.np(_mb.dt.float8e4)
    PT8 = np.zeros((NDEV, 128, nchunk * 128), f8)
    for dv in range(NDEV):
        S = SLOT[dv].astype(np.int32)          # [128 e, nchunk k]
        e_i, k_i = np.nonzero(S < 128)
        PT8[dv][S[e_i, k_i], k_i * 128 + e_i] = 1.0

    in_maps = []
    for dv in range(NDEV):
        in_maps.append(
            {
                "xT": np.ascontiguousarray(xpT[dv]),
                "srcw": np.ascontiguousarray(srcw[dv]),
                "slotf": np.ascontiguousarray(SLOT[dv]),
                "wfull": wfull,
                "w3s": W3s,
                "binv5": binv5,
                "b5c": b5c,
                "iotaf": iotaf,
                "pt8": PT8[dv],
            }
        )
    return in_maps, pos, cpb, nchunk


# ---------------- bass program ----------------
def build(cfg, cpb, nchunk):
    import concourse.bass as bass
    import concourse.bacc as bacc
    import concourse.tile as tile
    from concourse import mybir
    from concourse.masks import make_identity

    f32 = mybir.dt.float32
    f32r = mybir.dt.float32r
    bf16 = mybir.dt.bfloat16
    i16 = mybir.dt.int16
    AF = mybir.ActivationFunctionType
    OP = mybir.AluOpType
    npd, bpd = cfg.npd, cfg.bpd
    ng = nchunk // GC

    nc = bacc.Bacc(
        "TRN2",
        target_bir_lowering=False,
        debug=False,
        enable_asserts=False,
        num_devices=NDEV,
    )
    xT = nc.dram_tensor("xT", [256, npd], f32, kind="ExternalInput").ap()
    wcols = GC * 128 // 16
    srcw = nc.dram_tensor("srcw", [128, wcols * ng], i16, kind="ExternalInput").ap()
    slotf = nc.dram_tensor("slotf", [128, nchunk], f32, kind="ExternalInput").ap()
    pt8 = nc.dram_tensor("pt8", [128, nchunk * 128], mybir.dt.float8e4,
                         kind="ExternalInput").ap()
    wfull = nc.dram_tensor("wfull", [6, 257, 258], f32, kind="ExternalInput").ap()
    w3s = nc.dram_tensor("w3s", [256, 256], f32, kind="ExternalInput").ap()
    binv5 = nc.dram_tensor("binv5", [128, 2, 2, 128], f32, kind="ExternalInput").ap()
    b5c = nc.dram_tensor("b5c", [128, 2], f32, kind="ExternalInput").ap()
    iotaf = nc.dram_tensor("iotaf", [128, 128], f32, kind="ExternalInput").ap()
    out = nc.dram_tensor("out", [npd, 256], f32, kind="ExternalOutput").ap()

    with tile.TileContext(nc) as tc:
        with (
            tc.tile_pool(name="cp", bufs=1) as cp,
            tc.tile_pool(name="sb", bufs=2) as sb,
            tc.tile_pool(name="gp", bufs=5) as gp,
            tc.tile_pool(name="psA", bufs=3, space="PSUM") as psA,
            tc.tile_pool(name="psB", bufs=2, space="PSUM") as psB,
            tc.tile_pool(name="dp", bufs=1, space="DRAM") as dp,
        ):
            # ---- constants ----
            ident = cp.tile([128, 128], f32)
            make_identity(nc, ident[:])
            iota_f = cp.tile([128, 128], f32)
            nc.sync.dma_start(iota_f[:], iotaf)
            iota_b = cp.tile([128, 128], bf16)
            nc.vector.tensor_copy(iota_b[:], iota_f[:])
            # layer-invariant edge indexing data, resident in SBUF
            pt8_sb = cp.tile([128, nchunk * 128], mybir.dt.float8e4)
            nc.sync.dma_start(pt8_sb[:], pt8)
            srcw_all = cp.tile([128, wcols * ng], i16)
            nc.sync.dma_start(srcw_all[:], srcw)
            slot_all = cp.tile([128, nchunk], f32)
            nc.sync.dma_start(slot_all[:], slotf)
            ones1f = cp.tile([1, 128], f32)
            nc.gpsimd.memset(ones1f[:], 1.0)
            ones1 = cp.tile([1, 128], f32r)
            nc.vector.tensor_copy(ones1[:], ones1f[:])
            onecol = cp.tile([128, 1], bf16)
            nc.gpsimd.memset(onecol[:], 1.0)
            b5sb = cp.tile([128, 2], f32)
            nc.sync.dma_start(b5sb[:], b5c)
            # stage fp32 loads through a small buffer, DVE-cast to f32r
            wextsb = cp.tile([128, 6, 2, 258], f32r)
            bwsb = cp.tile([1, 6, 258], f32r)
            w3ssb = cp.tile([128, 2, 256], f32r)
            binv5sb = cp.tile([128, 2, 2, 128], f32r)

            def _stage(dst_ap, src_ap, shape):
                st = sb.tile(list(shape), f32, tag="wstage", bufs=2, name="wstage")
                nc.sync.dma_start(st[:], src_ap)
                nc.vector.tensor_copy(dst_ap, st[:])

            for l in range(6):
                _stage(
                    wextsb[:, l],
                    wfull[l, 0:256, :].rearrange("(a p) c -> p a c", p=128),
                    [128, 2, 258],
                )
            _stage(bwsb[:], wfull[:, 256:257, :].rearrange("l o c -> o l c"),
                   [1, 6, 258])
            _stage(w3ssb[:], w3s.rearrange("(a p) m -> p a m", p=128), [128, 2, 256])
            _stage(binv5sb[:], binv5, [128, 2, 2, 128])

            # ---- DRAM comm buffers (per layer: Shared tensors allow one writer) ----
            tbl_owns = [dp.tile([npd, RW], bf16, name=f"tbl_own{i}") for i in range(6)]
            tbl_fulls = [
                dp.tile([NDEV * npd, RW], bf16, addr_space="Shared", name=f"tbl_full{i}")
                for i in range(6)
            ]

            # ---- layer-0 hT is just xT (W1@W2 folded into wfull[0]) ----
            hT = sb.tile([128, 2, npd], f32r, tag="hT")
            xr = xT.rearrange("(a p) n -> p a n", p=128)
            xstep = min(512, npd)
            for n0 in range(0, npd, xstep):
                _stage(hT[:, :, n0:n0 + xstep], xr[:, :, n0:n0 + xstep],
                       [128, 2, xstep])

            for l in range(6):
                tbl_own, tbl_full = tbl_owns[l], tbl_fulls[l]
                # ---- own-node hw' + table build ----
                tbl_sb = sb.tile([128, bpd, RW], bf16, tag="tbl")
                edst_all = sb.tile([128, bpd], bf16, tag="edst")
                for b in range(bpd):
                    pshw = psB.tile([128, 258], f32, tag="pshw")
                    for ki in range(2):
                        nc.tensor.matmul(
                            pshw[:],
                            lhsT=hT[:, ki, b * 128 : (b + 1) * 128],
                            rhs=wextsb[:, l, ki, :],
                            start=(ki == 0),
                            stop=False,
                        )
                    nc.tensor.matmul(
                        pshw[:],
                        lhsT=ones1[:],
                        rhs=bwsb[:, l, :],
                        start=False,
                        stop=True,
                    )
                    nc.vector.tensor_copy(tbl_sb[:, b, 0:256], pshw[:, 0:256])
                    nc.vector.tensor_copy(edst_all[:, b : b + 1], pshw[:, 256:257])
                nc.sync.dma_start(
                    tbl_own[:].rearrange("(b p) c -> p b c", p=128), tbl_sb[:]
                )
                if AG_MODE == "tiny":
                    # timing probe: near-zero-byte collective, same structure
                    nc.gpsimd.collective_compute(
                        "AllGather",
                        mybir.AluOpType.bypass,
                        replica_groups=[list(range(NDEV))],
                        ins=[tbl_own[0:16]],
                        outs=[tbl_full[0:128]],
                    )
                else:
                    nc.gpsimd.collective_compute(
                        "AllGather",
                        mybir.AluOpType.bypass,
                        replica_groups=[list(range(NDEV))],
                        ins=[tbl_own[:]],
                        outs=[tbl_full[:]],
                    )

                # ---- e_dst per edge for ALL groups (overlaps the AllGather:
                # depends only on edst_all + the static one-hot transpose) ----
                EB_all = sb.tile([128, nchunk], f32, tag="eball")
                for g in range(ng):
                    pseb = psB.tile([128, GC], f32, tag="pseb", bufs=2)
                    for c in range(GC):
                        k = g * GC + c
                        b = k // cpb
                        nc.tensor.matmul(
                            pseb[:, c : c + 1],
                            lhsT=pt8_sb[:, k * 128:(k + 1) * 128],
                            rhs=edst_all[:, b : b + 1],
                            start=True,
                            stop=True,
                        )
                    nc.vector.tensor_copy(EB_all[:, g * GC:(g + 1) * GC], pseb[:])

                # ---- gather / scatter ----
                hT_next = sb.tile([128, 2, npd], f32r, tag="hT")
                pss = None
                for g in range(ng):
                    # --- group prologue: src gather + e_dst broadcast + ee ---
                    G = gp.tile([128, GC, RW], bf16, tag="G")
                    if SKIP != "gather":
                        nc.gpsimd.dma_gather(
                            out_ap=G[:], in_ap=tbl_full[:],
                            idxs_ap=srcw_all[:, g * wcols:(g + 1) * wcols],
                            num_idxs=GC * 128, num_idxs_reg=GC * 128,
                            elem_size=RW,
                        )
                    X = gp.tile([128, 2 * GC], f32, tag="X")
                    nc.vector.tensor_tensor(
                        X[:, 0:GC], G[:, :, 0],
                        EB_all[:, g * GC:(g + 1) * GC], op=OP.add
                    )
                    nc.vector.tensor_scalar(
                        out=X[:, GC:2 * GC], in0=X[:, 0:GC], scalar1=NEG,
                        scalar2=None, op0=OP.mult,
                    )
                    E1 = gp.tile([128, 2 * GC], f32, tag="E1")
                    nc.scalar.activation(E1[:], X[:], AF.Exp)
                    EE = gp.tile([128, GC], f32, tag="EE")
                    nc.vector.tensor_tensor(
                        EE[:], E1[:, 0:GC], E1[:, GC:2 * GC], op=OP.max
                    )
                    # --- scatter pass ---
                    for c in range(GC):
                        k = g * GC + c
                        b, cc = divmod(k, cpb)
                        if cc == 0:
                            pss = psA.tile([128, 257], f32, tag="pss")
                        lt = gp.tile([128, 128], bf16, tag="lt")
                        nc.vector.tensor_scalar(
                            out=lt[:],
                            in0=iota_b[:],
                            scalar1=slot_all[:, k : k + 1],
                            scalar2=EE[:, c : c + 1],
                            op0=OP.is_equal,
                            op1=OP.mult,
                        )
                        nc.tensor.matmul(
                            pss[:, 0:256],
                            lhsT=lt[:],
                            rhs=G[:, c, :],
                            start=(cc == 0),
                            stop=(cc == cpb - 1),
                        )
                        # denominator column: same stationary lt, ones rhs.
                        # start=False always: the message matmul's start=True
                        # already cleared the whole bank's has_written bits
                        # (a second start here would re-clear them and drop
                        # chunk 0's messages); col 256's bit is clear, so the
                        # first write overwrites, later ones accumulate.
                        nc.tensor.matmul(
                            pss[:, 256:257],
                            lhsT=lt[:],
                            rhs=onecol[:],
                            start=False,
                            stop=(cc == cpb - 1),
                        )
                        if cc != cpb - 1:
                            continue
                        # ---- block epilogue: normalize ----
                        den = sb.tile([128, 1], f32, tag="den")
                        nc.vector.tensor_scalar(
                            out=den[:], in0=pss[:, 256:257], scalar1=1e-30,
                            scalar2=None, op0=OP.add,
                        )
                        rec = sb.tile([128, 1], f32, tag="rec")
                        nc.vector.reciprocal(rec[:], den[:])
                        onrm = sb.tile([128, 256], f32, tag="onrm")
                        nc.scalar.activation(
                            onrm[:], pss[:, 0:256], AF.Copy, scale=rec[:]
                        )
                        srct_t = onrm
                        pst = psB.tile([128, 256], f32, tag="pst", bufs=1)
                        for hh in range(2):
                            nc.tensor.transpose(
                                out=pst[:, hh * 128 : (hh + 1) * 128],
                                in_=srct_t[:, hh * 128 : (hh + 1) * 128],
                                identity=ident[:],
                            )
                            nc.vector.tensor_copy(
                                hT_next[:, hh, b * 128 : (b + 1) * 128],
                                pst[:, hh * 128 : (hh + 1) * 128],
                            )
                hT = hT_next

            # ---- final: h6 = n'5 @ B5^-1 + b5; out = relu(h6) @ (W3_top+W3_bot)
            # batch 4 node blocks per output DMA: fewer HWDGE dispatches
            OB = 4
            for b0 in range(0, bpd, OB):
                nb = min(OB, bpd - b0)
                oo = sb.tile([128, OB, 256], f32, tag="oo")
                for bi in range(nb):
                    b = b0 + bi
                    psf2 = psB.tile([128, 256], f32, tag="pst", bufs=1)
                    for dj in range(2):
                        for ki in range(2):
                            nc.tensor.matmul(
                                psf2[:, dj * 128 : (dj + 1) * 128],
                                lhsT=binv5sb[:, ki, dj],
                                rhs=hT[:, ki, b * 128 : (b + 1) * 128],
                                start=(ki == 0),
                                stop=(ki == 1),
                            )
                    zT = sb.tile([128, 2, 128], f32r, tag="zT")
                    for hh in range(2):
                        nc.vector.tensor_scalar(
                            out=zT[:, hh],
                            in0=psf2[:, hh * 128 : (hh + 1) * 128],
                            scalar1=b5sb[:, hh : hh + 1],
                            scalar2=0.0,
                            op0=OP.add,
                            op1=OP.max,
                        )
                    psf = psB.tile([128, 256], f32, tag="pshw")
                    for ki in range(2):
                        nc.tensor.matmul(
                            psf[:],
                            lhsT=zT[:, ki],
                            rhs=w3ssb[:, ki, :],
                            start=(ki == 0),
                            stop=(ki == 1),
                        )
                    nc.vector.tensor_copy(oo[:, bi, :], psf[:])
                nc.sync.dma_start(
                    out[b0 * 128 : (b0 + nb) * 128, :].rearrange(
                        "(b p) c -> p b c", p=128
                    ),
                    oo[:, 0:nb, :],
                )

    nc.compile()
    return nc


# ---------------- entry point ----------------
def kernel(**inputs):
    cfg = FULL
    in_maps, pos, cpb, nchunk = prep(inputs, cfg)
    nc = build(cfg, cpb, nchunk)
    from concourse import bass_utils

    res = bass_utils.run_bass_kernel_spmd(nc, in_maps, core_ids=list(range(NDEV)))
    outs = [res.results[dv]["out"] for dv in range(NDEV)]
    full = np.zeros((cfg.n, 256), np.float32)
    full[:] = np.stack(outs).reshape(NDEV * cfg.npd, 256)[pos]
    return full



# revision 21
# speedup vs baseline: 1.4202x; 1.0445x over previous
"""Trainium2 Bass kernel for a 6-layer GAT GNN (nn_GAT_GNN_35579509080109).

Strategy (8 NeuronCores, node partition):
  - Nodes are degree-balanced into 160 blocks of 128 slots (125 real nodes
    each); each device owns 20 blocks (2560 padded node slots).
  - Per layer, each device computes hw = h @ [W | W a_src | W a_dst] (+ folded
    bias) for its own nodes, packs a 768B row table
    [hw(256) bf16 | 1.0 | pad | e_src f32 | e_dst f32 | pad..384], and
    AllGathers the table. The front MLP (W1@W2) is folded into layer 0's
    weights, so layer 0 consumes xT directly.
  - Edges are partitioned by destination owner, sorted into dst blocks, and
    processed in chunks of 128 edges: hw[src]+e_src via a batched dma_gather
    (768B rows, by global src row). Per-edge index streams (srcw, slotf) are
    layer-invariant and stay resident in SBUF (loaded once).
  - e_dst per edge comes from a host-precomputed static one-hot transpose
    ptT[slot, edge] (fp8, SBUF-resident, 5.2MB): eb = ptT^T @ e_dst_col per
    chunk on the tensor engine (no aux dma_gather). These broadcasts depend
    only on local hw, so they overlap the AllGather.
  - Per-edge softmax numerators ee = exp(leaky_relu(e_src+e_dst)) computed as
    max(exp(x), exp(0.2 x)) on the scalar engine in [128, GC] batches.
  - Scatter-add + denominators on the tensor engine: one-hot(dst slot) * ee
    (lhsT, bf16) x [hw | 1] (rhs) accumulates [128 dst, 257] in PSUM per
    block; out = psum[:, :256] / psum[:, 256].
  - Layer bias folds into the next layer's matmul (extra K=1 row); the final
    concat(h, h) @ W3 collapses to relu(h) @ (W3_top + W3_bot).

Notes from tuning (vs the earlier baseline: sim 2.64ms -> 2.38ms; paired
real-HW A/B deltas -1.1ms and -2.6ms/call across two sessions; rel err
1.158e-3 vs the 2e-2 gate):
  - The 256B-row aux dma_gather for e_dst was ~0.44ms of DMA busy across the
    run; replaced by the static fp8 ptT matmuls (PE has headroom).
  - Real HW punishes small-DMA dispatch overhead 4-10x beyond the cost
    model; that is why the aux-gather removal, persistent index streams,
    and the batched (4-blocks-per-DMA) output writes out-performed their
    sim deltas.
  - fp8 (e4m3) for the 256-value hw payload works mechanically (512B rows,
    with per-layer scaling to dodge underflow: activations decay ~5x/layer
    and unscaled values silently flush to zero) but lands at rel err 1.7e-2
    vs the 2e-2 gate - too close; kept bf16.
  - Do NOT exceed 1024 indices per dma_gather: a 2048-idx build compiles
    and simulates cleanly but hangs the device at first execution.
  - remote_dma_broadcast + tc.Switch would replace the 162us/layer (real)
    AllGather with overlapped p2p writes, but the Tile scheduler's no_exec
    cost model cannot deliver remote sem updates ("known gap"), so any
    receiver-side wait deadlocks schedule_block. Dead end in this build.
  - Also closed: strided/sliced collective APs (NEFF compile reject),
    split/hierarchical/AllReduce/AllToAll exchanges (size-cost curve),
    layer-fusion code motion (+13us PE/PSUM contention), deeper buffer
    rotations (SBUF/PSUM at capacity). Remaining floors: the AllGather and
    the 768B/edge gather stride (256B-multiple HW minimum for hw+e_src).
"""
import os
import sys
import numpy as np

for _p in ("/opt/trn_rl_repo", "/root/.axon_site/_ro/trn_rl_repo"):
    if os.path.isdir(_p) and _p not in sys.path:
        sys.path.append(_p)

# ---------------- problem constants ----------------
N = 20000
E = 320000
D = 256
NEG = 0.2
NDEV = 8

GC = 8    # chunks per gather group (1024 edges / dma_gather call; HW limit ~1024 idxs)
RW = 256  # table row width in bf16 (512 bytes): rotated hw' only
# timing-probe knobs (correctness only guaranteed for defaults)
AG_MODE = os.environ.get("KAG", "full")
SKIP = os.environ.get("KSKIP", "")


class Cfg:
    def __init__(self, n, e, bpd):
        self.n, self.e, self.bpd = n, e, bpd
        self.npd = bpd * 128
        self.nblk = NDEV * bpd

FULL = Cfg(N, E, 20)


def _wrap16(flat):
    """dma_gather index layout: idx i at [i%16, i//16], replicated to 128 rows."""
    ni = flat.shape[0]
    w = np.ascontiguousarray(flat.reshape(ni // 16, 16).T).astype(np.int16)
    return np.tile(w, (8, 1))


# ---------------- host preprocessing ----------------
def prep(inputs, cfg):
    x = np.ascontiguousarray(np.asarray(inputs["x"], np.float32))
    ei = np.asarray(inputs["edge_index"]).astype(np.int64)
    W1 = np.asarray(inputs["W1"], np.float32)
    W2 = np.asarray(inputs["W2"], np.float32)
    Ws = np.asarray(inputs["Ws"], np.float32)
    a_src = np.asarray(inputs["a_src"], np.float32)
    a_dst = np.asarray(inputs["a_dst"], np.float32)
    bias = np.asarray(inputs["bias"], np.float32)
    W3 = np.asarray(inputs["W3"], np.float32)
    src, dst = ei[0], ei[1]
    n, bpd, npd, nblk = cfg.n, cfg.bpd, cfg.npd, cfg.nblk

    # --- degree-balanced node -> (dev, blk, slot) assignment (snake) ---
    deg = np.bincount(dst, minlength=n)
    order = np.argsort(-deg, kind="stable")
    r = np.arange(n)
    stripe = r // nblk
    posin = r % nblk
    blk_glob = np.where(stripe % 2 == 0, posin, nblk - 1 - posin)
    slot = stripe
    assert slot.max() < 128
    pos = np.empty(n, np.int64)
    pos[order] = (blk_glob // bpd) * npd + (blk_glob % bpd) * 128 + slot

    # --- edge grouping by dst block ---
    dstp = pos[dst]
    bid = dstp // npd * bpd + (dstp % npd) // 128  # global block id
    sidx = np.argsort(bid, kind="stable")
    counts = np.bincount(bid, minlength=nblk)
    cpb = int(np.ceil(counts.max() / 128))
    nchunk_raw = bpd * cpb
    nchunk = ((nchunk_raw + GC - 1) // GC) * GC
    starts = np.zeros(nblk + 1, np.int64)
    starts[1:] = np.cumsum(counts)
    rank = np.arange(cfg.e) - starts[bid[sidx]]

    sdev = (dstp // npd)[sidx]
    sblk = ((dstp % npd) // 128)[sidx]
    sslot = (dstp % 128)[sidx]
    ssrc = pos[src][sidx]
    kk = sblk * cpb + rank // 128
    pp = rank % 128

    SRC = np.zeros((NDEV, 128, nchunk), np.int32)       # global table row of src
    SLOT = np.full((NDEV, 128, nchunk), 255.0, np.float32)
    SRC[sdev, pp, kk] = ssrc
    SLOT[sdev, pp, kk] = sslot

    # wrapped int16 index arrays for dma_gather, per group of GC chunks
    ng = nchunk // GC
    wcols = GC * 128 // 16
    srcw = np.zeros((NDEV, 128, wcols * ng), np.int16)
    for dv in range(NDEV):
        for g in range(ng):
            # edge i in group = c*128 + p, c in [0,GC)
            flat_s = SRC[dv][:, g * GC:(g + 1) * GC].T.reshape(-1)  # [GC*128] c-major
            srcw[dv][:, g * wcols:(g + 1) * wcols] = _wrap16(flat_s)

    # --- x permuted / padded / transposed ---
    xp = np.zeros((NDEV, npd, D), np.float32)
    xp[pos // npd, pos % npd] = x
    xpT = np.ascontiguousarray(xp.transpose(0, 2, 1))

    # --- weights: per-layer rotation folds e_src into hw'[:, 0] ---
    # B'_l = H_l @ diag(||a_src_l||, 1, ...) with Householder H_l e1 =
    # a_src_l/||a_src_l||, so (h @ W_l @ B'_l)[:, 0] = h @ W_l @ a_src_l =
    # e_src and the rest is an orthogonal rotation of hw. The table row is
    # then just 256 bf16 (512B): no separate e_src, no ones column. B'^{-1}
    # folds into layer l+1's weights host-side; layer 5's inverse is applied
    # on-device in the final matmul (binv5).
    W12 = np.ascontiguousarray(W1 @ W2)
    wfull = np.zeros((6, 257, 258), np.float32)
    Binv_prev = None
    for l in range(6):
        u = a_src[l].astype(np.float64)
        nu = float(np.linalg.norm(u))
        e1 = np.zeros(256, np.float64)
        e1[0] = 1.0
        v = u / nu - e1
        vv = float(v @ v)
        H = np.eye(256) - (2.0 / vv) * np.outer(v, v) if vv > 1e-12 else np.eye(256)
        Bp = H.copy()
        Bp[:, 0] *= nu
        Binv = H.copy()
        Binv[0, :] /= nu
        wext = np.concatenate(
            [Ws[l].astype(np.float64) @ Bp,
             (Ws[l] @ a_dst[l]).astype(np.float64)[:, None],
             np.zeros((256, 1))], axis=1
        )  # zero pad col: fp32r matmul needs even free width
        # layer 0 consumes x directly: fold the front MLP (W1 @ W2) in
        if l == 0:
            wfull[l, :256] = (W12.astype(np.float64) @ wext).astype(np.float32)
        else:
            wfull[l, :256] = (Binv_prev @ wext).astype(np.float32)
            wfull[l, 256] = (bias[l - 1].astype(np.float64) @ wext).astype(np.float32)
        Binv_prev = Binv
    W3s = np.ascontiguousarray(W3[:256] + W3[256:])
    binv5 = np.ascontiguousarray(
        Binv_prev.reshape(2, 128, 2, 128).transpose(1, 0, 2, 3)
    ).astype(np.float32)
    b5c = np.ascontiguousarray(bias[5].reshape(2, 128).T).astype(np.float32)
    iotaf = np.tile(np.arange(128, dtype=np.float32)[None, :], (128, 1))
    # static one-hot transpose per chunk: ptT[slot, k*128+e] = (slot(e,k) == slot)
    from concourse import mybir as _mb
    f8 = _mb.dt.np(_mb.dt.float8e4)
    PT8 = np.zeros((NDEV, 128, nchunk * 128), f8)
    for dv in range(NDEV):
        S = SLOT[dv].astype(np.int32)          # [128 e, nchunk k]
        e_i, k_i = np.nonzero(S < 128)
        PT8[dv][S[e_i, k_i], k_i * 128 + e_i] = 1.0

    in_maps = []
    for dv in range(NDEV):
        in_maps.append(
            {
                "xT": np.ascontiguousarray(xpT[dv]),
                "srcw": np.ascontiguousarray(srcw[dv]),
                "slotf": np.ascontiguousarray(SLOT[dv]),
                "wfull": wfull,
                "w3s": W3s,
                "binv5": binv5,
                "b5c": b5c,
                "iotaf": iotaf,
                "pt8": PT8[dv],
            }
        )
    return in_maps, pos, cpb, nchunk


# ---------------- bass program ----------------
def build(cfg, cpb, nchunk):
    import concourse.bass as bass
    import concourse.bacc as bacc
    import concourse.tile as tile
    from concourse import mybir
    from concourse.masks import make_identity

    f32 = mybir.dt.float32
    f32r = mybir.dt.float32r
    bf16 = mybir.dt.bfloat16
    i16 = mybir.dt.int16
    AF = mybir.ActivationFunctionType
    OP = mybir.AluOpType
    npd, bpd = cfg.npd, cfg.bpd
    ng = nchunk // GC

    nc = bacc.Bacc(
        "TRN2",
        target_bir_lowering=False,
        debug=False,
        enable_asserts=False,
        num_devices=NDEV,
        num_swdge_queues=4,
    )
    xT = nc.dram_tensor("xT", [256, npd], f32, kind="ExternalInput").ap()
    wcols = GC * 128 // 16
    srcw = nc.dram_tensor("srcw", [128, wcols * ng], i16, kind="ExternalInput").ap()
    slotf = nc.dram_tensor("slotf", [128, nchunk], f32, kind="ExternalInput").ap()
    pt8 = nc.dram_tensor("pt8", [128, nchunk * 128], mybir.dt.float8e4,
                         kind="ExternalInput").ap()
    wfull = nc.dram_tensor("wfull", [6, 257, 258], f32, kind="ExternalInput").ap()
    w3s = nc.dram_tensor("w3s", [256, 256], f32, kind="ExternalInput").ap()
    binv5 = nc.dram_tensor("binv5", [128, 2, 2, 128], f32, kind="ExternalInput").ap()
    b5c = nc.dram_tensor("b5c", [128, 2], f32, kind="ExternalInput").ap()
    iotaf = nc.dram_tensor("iotaf", [128, 128], f32, kind="ExternalInput").ap()
    out = nc.dram_tensor("out", [npd, 256], f32, kind="ExternalOutput").ap()

    with tile.TileContext(nc) as tc:
        with (
            tc.tile_pool(name="cp", bufs=1) as cp,
            tc.tile_pool(name="sb", bufs=2) as sb,
            tc.tile_pool(name="gp", bufs=8) as gp,
            tc.tile_pool(name="psA", bufs=3, space="PSUM") as psA,
            tc.tile_pool(name="psB", bufs=2, space="PSUM") as psB,
            tc.tile_pool(name="dp", bufs=1, space="DRAM") as dp,
        ):
            # ---- constants ----
            ident = cp.tile([128, 128], f32)
            make_identity(nc, ident[:])
            iota_f = cp.tile([128, 128], f32)
            nc.sync.dma_start(iota_f[:], iotaf)
            iota_b = cp.tile([128, 128], bf16)
            nc.vector.tensor_copy(iota_b[:], iota_f[:])
            # layer-invariant edge indexing data, resident in SBUF
            pt8_sb = cp.tile([128, nchunk * 128], mybir.dt.float8e4)
            nc.sync.dma_start(pt8_sb[:], pt8)
            srcw_all = cp.tile([128, wcols * ng], i16)
            nc.sync.dma_start(srcw_all[:], srcw)
            slot_all = cp.tile([128, nchunk], f32)
            nc.sync.dma_start(slot_all[:], slotf)
            ones1f = cp.tile([1, 128], f32)
            nc.gpsimd.memset(ones1f[:], 1.0)
            ones1 = cp.tile([1, 128], f32r)
            nc.vector.tensor_copy(ones1[:], ones1f[:])
            onecol = cp.tile([128, 1], bf16)
            nc.gpsimd.memset(onecol[:], 1.0)
            b5sb = cp.tile([128, 2], f32)
            nc.sync.dma_start(b5sb[:], b5c)
            # stage fp32 loads through a small buffer, DVE-cast to f32r
            wextsb = cp.tile([128, 6, 2, 258], f32r)
            bwsb = cp.tile([1, 6, 258], f32r)
            w3ssb = cp.tile([128, 2, 256], f32r)
            binv5sb = cp.tile([128, 2, 2, 128], f32r)

            def _stage(dst_ap, src_ap, shape):
                st = sb.tile(list(shape), f32, tag="wstage", bufs=2, name="wstage")
                nc.sync.dma_start(st[:], src_ap)
                nc.vector.tensor_copy(dst_ap, st[:])

            for l in range(6):
                _stage(
                    wextsb[:, l],
                    wfull[l, 0:256, :].rearrange("(a p) c -> p a c", p=128),
                    [128, 2, 258],
                )
            _stage(bwsb[:], wfull[:, 256:257, :].rearrange("l o c -> o l c"),
                   [1, 6, 258])
            _stage(w3ssb[:], w3s.rearrange("(a p) m -> p a m", p=128), [128, 2, 256])
            _stage(binv5sb[:], binv5, [128, 2, 2, 128])

            # ---- DRAM comm buffers (per layer: Shared tensors allow one writer) ----
            tbl_owns = [dp.tile([npd, RW], bf16, name=f"tbl_own{i}") for i in range(6)]
            tbl_fulls = [
                dp.tile([NDEV * npd, RW], bf16, addr_space="Shared", name=f"tbl_full{i}")
                for i in range(6)
            ]

            # ---- layer-0 hT is just xT (W1@W2 folded into wfull[0]) ----
            hT = sb.tile([128, 2, npd], f32r, tag="hT")
            xr = xT.rearrange("(a p) n -> p a n", p=128)
            xstep = min(512, npd)
            for n0 in range(0, npd, xstep):
                _stage(hT[:, :, n0:n0 + xstep], xr[:, :, n0:n0 + xstep],
                       [128, 2, xstep])

            for l in range(6):
                tbl_own, tbl_full = tbl_owns[l], tbl_fulls[l]
                # ---- own-node hw' + table build ----
                tbl_sb = sb.tile([128, bpd, RW], bf16, tag="tbl")
                edst_all = sb.tile([128, bpd], bf16, tag="edst")
                for b in range(bpd):
                    pshw = psB.tile([128, 258], f32, tag="pshw")
                    for ki in range(2):
                        nc.tensor.matmul(
                            pshw[:],
                            lhsT=hT[:, ki, b * 128 : (b + 1) * 128],
                            rhs=wextsb[:, l, ki, :],
                            start=(ki == 0),
                            stop=False,
                        )
                    nc.tensor.matmul(
                        pshw[:],
                        lhsT=ones1[:],
                        rhs=bwsb[:, l, :],
                        start=False,
                        stop=True,
                    )
                    nc.vector.tensor_copy(tbl_sb[:, b, 0:256], pshw[:, 0:256])
                    nc.vector.tensor_copy(edst_all[:, b : b + 1], pshw[:, 256:257])
                nc.sync.dma_start(
                    tbl_own[:].rearrange("(b p) c -> p b c", p=128), tbl_sb[:]
                )
                if AG_MODE == "tiny":
                    # timing probe: near-zero-byte collective, same structure
                    nc.gpsimd.collective_compute(
                        "AllGather",
                        mybir.AluOpType.bypass,
                        replica_groups=[list(range(NDEV))],
                        ins=[tbl_own[0:16]],
                        outs=[tbl_full[0:128]],
                    )
                else:
                    nc.gpsimd.collective_compute(
                        "AllGather",
                        mybir.AluOpType.bypass,
                        replica_groups=[list(range(NDEV))],
                        ins=[tbl_own[:]],
                        outs=[tbl_full[:]],
                    )

                # ---- e_dst per edge for ALL groups (overlaps the AllGather:
                # depends only on edst_all + the static one-hot transpose) ----
                EB_all = sb.tile([128, nchunk], f32, tag="eball")
                for g in range(ng):
                    pseb = psB.tile([128, GC], f32, tag="pseb", bufs=2)
                    for c in range(GC):
                        k = g * GC + c
                        b = k // cpb
                        nc.tensor.matmul(
                            pseb[:, c : c + 1],
                            lhsT=pt8_sb[:, k * 128:(k + 1) * 128],
                            rhs=edst_all[:, b : b + 1],
                            start=True,
                            stop=True,
                        )
                    nc.vector.tensor_copy(EB_all[:, g * GC:(g + 1) * GC], pseb[:])

                # ---- gather / scatter ----
                hT_next = sb.tile([128, 2, npd], f32r, tag="hT")
                pss = None
                for g in range(ng):
                    # --- group prologue: src gather + e_dst broadcast + ee ---
                    G = gp.tile([128, GC, RW], bf16, tag="G")
                    if SKIP != "gather":
                        nc.gpsimd.dma_gather(
                            out_ap=G[:], in_ap=tbl_full[:],
                            idxs_ap=srcw_all[:, g * wcols:(g + 1) * wcols],
                            num_idxs=GC * 128, num_idxs_reg=GC * 128,
                            elem_size=RW,
                            queue_num=g % 4,
                        )
                    X = gp.tile([128, 2 * GC], f32, tag="X")
                    nc.vector.tensor_tensor(
                        X[:, 0:GC], G[:, :, 0],
                        EB_all[:, g * GC:(g + 1) * GC], op=OP.add
                    )
                    nc.vector.tensor_scalar(
                        out=X[:, GC:2 * GC], in0=X[:, 0:GC], scalar1=NEG,
                        scalar2=None, op0=OP.mult,
                    )
                    E1 = gp.tile([128, 2 * GC], f32, tag="E1")
                    nc.scalar.activation(E1[:], X[:], AF.Exp)
                    EE = gp.tile([128, GC], f32, tag="EE")
                    nc.vector.tensor_tensor(
                        EE[:], E1[:, 0:GC], E1[:, GC:2 * GC], op=OP.max
                    )
                    # --- scatter pass ---
                    for c in range(GC):
                        k = g * GC + c
                        b, cc = divmod(k, cpb)
                        if cc == 0:
                            pss = psA.tile([128, 257], f32, tag="pss")
                        lt = gp.tile([128, 128], bf16, tag="lt")
                        nc.vector.tensor_scalar(
                            out=lt[:],
                            in0=iota_b[:],
                            scalar1=slot_all[:, k : k + 1],
                            scalar2=EE[:, c : c + 1],
                            op0=OP.is_equal,
                            op1=OP.mult,
                        )
                        nc.tensor.matmul(
                            pss[:, 0:256],
                            lhsT=lt[:],
                            rhs=G[:, c, :],
                            start=(cc == 0),
                            stop=(cc == cpb - 1),
                        )
                        # denominator column: same stationary lt, ones rhs.
                        # start=False always: the message matmul's start=True
                        # already cleared the whole bank's has_written bits
                        # (a second start here would re-clear them and drop
                        # chunk 0's messages); col 256's bit is clear, so the
                        # first write overwrites, later ones accumulate.
                        nc.tensor.matmul(
                            pss[:, 256:257],
                            lhsT=lt[:],
                            rhs=onecol[:],
                            start=False,
                            stop=(cc == cpb - 1),
                        )
                        if cc != cpb - 1:
                            continue
                        # ---- block epilogue: normalize ----
                        den = sb.tile([128, 1], f32, tag="den")
                        nc.vector.tensor_scalar(
                            out=den[:], in0=pss[:, 256:257], scalar1=1e-30,
                            scalar2=None, op0=OP.add,
                        )
                        rec = sb.tile([128, 1], f32, tag="rec")
                        nc.vector.reciprocal(rec[:], den[:])
                        onrm = sb.tile([128, 256], f32, tag="onrm")
                        nc.scalar.activation(
                            onrm[:], pss[:, 0:256], AF.Copy, scale=rec[:]
                        )
                        srct_t = onrm
                        pst = psB.tile([128, 256], f32, tag="pst", bufs=1)
                        for hh in range(2):
                            nc.tensor.transpose(
                                out=pst[:, hh * 128 : (hh + 1) * 128],
                                in_=srct_t[:, hh * 128 : (hh + 1) * 128],
                                identity=ident[:],
                            )
                            nc.vector.tensor_copy(
                                hT_next[:, hh, b * 128 : (b + 1) * 128],
                                pst[:, hh * 128 : (hh + 1) * 128],
                            )
                hT = hT_next

            # ---- final: h6 = n'5 @ B5^-1 + b5; out = relu(h6) @ (W3_top+W3_bot)
            # batch 4 node blocks per output DMA: fewer HWDGE dispatches
            OB = 4
            for b0 in range(0, bpd, OB):
                nb = min(OB, bpd - b0)
                oo = sb.tile([128, OB, 256], f32, tag="oo")
                for bi in range(nb):
                    b = b0 + bi
                    psf2 = psB.tile([128, 256], f32, tag="pst", bufs=1)
                    for dj in range(2):
                        for ki in range(2):
                            nc.tensor.matmul(
                                psf2[:, dj * 128 : (dj + 1) * 128],
                                lhsT=binv5sb[:, ki, dj],
                                rhs=hT[:, ki, b * 128 : (b + 1) * 128],
                                start=(ki == 0),
                                stop=(ki == 1),
                            )
                    zT = sb.tile([128, 2, 128], f32r, tag="zT")
                    for hh in range(2):
                        nc.vector.tensor_scalar(
                            out=zT[:, hh],
                            in0=psf2[:, hh * 128 : (hh + 1) * 128],
                            scalar1=b5sb[:, hh : hh + 1],
                            scalar2=0.0,
                            op0=OP.add,
                            op1=OP.max,
                        )
                    psf = psB.tile([128, 256], f32, tag="pshw")
                    for ki in range(2):
                        nc.tensor.matmul(
                            psf[:],
                            lhsT=zT[:, ki],
                            rhs=w3ssb[:, ki, :],
                            start=(ki == 0),
                            stop=(ki == 1),
                        )
                    nc.vector.tensor_copy(oo[:, bi, :], psf[:])
                nc.sync.dma_start(
                    out[b0 * 128 : (b0 + nb) * 128, :].rearrange(
                        "(b p) c -> p b c", p=128
                    ),
                    oo[:, 0:nb, :],
                )

    nc.compile()
    return nc


# ---------------- entry point ----------------
def kernel(**inputs):
    cfg = FULL
    in_maps, pos, cpb, nchunk = prep(inputs, cfg)
    nc = build(cfg, cpb, nchunk)
    from concourse import bass_utils

    res = bass_utils.run_bass_kernel_spmd(nc, in_maps, core_ids=list(range(NDEV)))
    outs = [res.results[dv]["out"] for dv in range(NDEV)]
    full = np.zeros((cfg.n, 256), np.float32)
    full[:] = np.stack(outs).reshape(NDEV * cfg.npd, 256)[pos]
    return full



# revision 26
# speedup vs baseline: 2.7514x; 1.9373x over previous
"""Trainium2 Bass kernel for a 6-layer GAT GNN (nn_GAT_GNN_35579509080109).

Strategy (8 NeuronCores, node partition):
  - Nodes are degree-balanced into 160 blocks of 128 slots (125 real nodes
    each); each device owns 20 blocks (2560 padded node slots).
  - Per layer, each device computes hw' = h @ (W_l @ B'_l) for its own nodes,
    where B'_l is a Householder rotation whose first column is a_src_l: so
    hw'[:, 0] IS e_src and the table row is just 256 bf16 = 512B (no separate
    e_src, no ones column). B'^{-1} folds into layer l+1's weights host-side
    (layer 5's inverse is applied on-device via binv5 in the final matmuls).
    The table is AllGathered (10.5MB out vs 15.7MB for the old 768B rows).
  - Edges are partitioned by destination owner, sorted into dst blocks, and
    processed in chunks of 128 edges: hw'[src] rows via dma_gather, issued as
    4x256-idx calls round-robined over 4 SWDGE queues (A/B-measured optimum;
    1x1024 on one queue is ~0.7ms/call slower end-to-end - SWDGE descriptor
    generation + completion latency is the dominant real-HW cost here).
    Per-edge index streams (srcw, slotf) are layer-invariant, SBUF-resident.
  - e_dst per edge comes from a host-precomputed static one-hot transpose
    ptT[slot, edge] (fp8, SBUF-resident, 5.2MB): eb = ptT^T @ e_dst_col per
    chunk on the tensor engine. Overlaps the AllGather issue.
  - ee = exp(leaky_relu(e_src+e_dst)) as max(exp(x), exp(0.2x)) on ScalarE.
  - Scatter-add on PE: lt = one-hot(dst slot)*ee (lhsT, bf16) x hw' rows
    (rhs, 256 wide) accumulates [128, 0:256] in PSUM per block; the
    denominator is a second 1-column matmul (same stationary lt, ones rhs)
    into psum[:, 256:257] with start=False ALWAYS - a start=True there would
    re-clear the whole PSUM bank's has_written bits and drop chunk 0's
    messages (hard-won lesson; rel err 0.34 until fixed).
  - Final: h6 = n'5 @ B5^{-1} + b5 on-device (relu can't fold through B), then
    out = relu(h6) @ (W3_top + W3_bot); output DMA batched 4 blocks/call.

Timing notes (this session; measured with drift-cancelling interleaved A/B
on real HW via axon PJRT, wall minus trivial-NEFF floor):
  - Baseline (768B rows, 1 SWDGE queue, gp bufs=5): ~2.45ms delta.
  - 512B rotated rows: -0.65ms (AllGather bytes -33%, gather bytes -33%).
  - 4 SWDGE queues + gp bufs=8: -0.30ms. gsplit 1024->2x512: -0.42ms;
    ->4x256: -0.15ms more; 8x128 regresses +0.26ms. gpbufs 11 ~= 8.
  - Net ~0.9-1.0ms delta vs trivial floor (~2.6x faster than baseline).
  - Tiny-AllGather probe: AG byte cost was ~113us/layer at 768B rows
    (~139GB/s effective); sim's 262us/layer collective model is ~2x high.
  - TimelineSim (trace=True + LazyPerfetto shims, see tlsim.py) showed zero
    compute overlapped the AllGather; the non-AG phase is where real HW ran
    ~2x over the cost model until the SWDGE parallelism fixes.

Older hard-won constraints that still hold:
  - Do NOT exceed 1024 indices per dma_gather (2048 hangs the device).
  - fp8 table payload lands at rel err 1.7e-2 vs the 2e-2 gate - too close.
  - remote_dma_broadcast receiver-side waits deadlock schedule_block.
  - Strided/sliced collective APs are a NEFF compile reject.
"""
import os
import sys
import numpy as np

for _p in ("/opt/trn_rl_repo", "/root/.axon_site/_ro/trn_rl_repo"):
    if os.path.isdir(_p) and _p not in sys.path:
        sys.path.append(_p)

# ---------------- problem constants ----------------
N = 20000
E = 320000
D = 256
NEG = 0.2
NDEV = 8

GC = 8    # chunks per gather group (1024 edges / dma_gather call; HW limit ~1024 idxs)
RW = 256  # table row width in bf16 (512 bytes): rotated hw' only
# timing-probe knobs (correctness only guaranteed for defaults)
AG_MODE = os.environ.get("KAG", "full")
SKIP = os.environ.get("KSKIP", "")


class Cfg:
    def __init__(self, n, e, bpd):
        self.n, self.e, self.bpd = n, e, bpd
        self.npd = bpd * 128
        self.nblk = NDEV * bpd

FULL = Cfg(N, E, 20)


def _wrap16(flat):
    """dma_gather index layout: idx i at [i%16, i//16], replicated to 128 rows."""
    ni = flat.shape[0]
    w = np.ascontiguousarray(flat.reshape(ni // 16, 16).T).astype(np.int16)
    return np.tile(w, (8, 1))


# ---------------- host preprocessing ----------------
def prep(inputs, cfg):
    x = np.ascontiguousarray(np.asarray(inputs["x"], np.float32))
    ei = np.asarray(inputs["edge_index"]).astype(np.int64)
    W1 = np.asarray(inputs["W1"], np.float32)
    W2 = np.asarray(inputs["W2"], np.float32)
    Ws = np.asarray(inputs["Ws"], np.float32)
    a_src = np.asarray(inputs["a_src"], np.float32)
    a_dst = np.asarray(inputs["a_dst"], np.float32)
    bias = np.asarray(inputs["bias"], np.float32)
    W3 = np.asarray(inputs["W3"], np.float32)
    src, dst = ei[0], ei[1]
    n, bpd, npd, nblk = cfg.n, cfg.bpd, cfg.npd, cfg.nblk

    # --- degree-balanced node -> (dev, blk, slot) assignment (snake) ---
    deg = np.bincount(dst, minlength=n)
    order = np.argsort(-deg, kind="stable")
    r = np.arange(n)
    stripe = r // nblk
    posin = r % nblk
    blk_glob = np.where(stripe % 2 == 0, posin, nblk - 1 - posin)
    slot = stripe
    assert slot.max() < 128
    pos = np.empty(n, np.int64)
    pos[order] = (blk_glob // bpd) * npd + (blk_glob % bpd) * 128 + slot

    # --- edge grouping by dst block ---
    dstp = pos[dst]
    bid = dstp // npd * bpd + (dstp % npd) // 128  # global block id
    sidx = np.argsort(bid, kind="stable")
    counts = np.bincount(bid, minlength=nblk)
    cpb = int(np.ceil(counts.max() / 128))
    nchunk_raw = bpd * cpb
    nchunk = ((nchunk_raw + GC - 1) // GC) * GC
    starts = np.zeros(nblk + 1, np.int64)
    starts[1:] = np.cumsum(counts)
    rank = np.arange(cfg.e) - starts[bid[sidx]]

    sdev = (dstp // npd)[sidx]
    sblk = ((dstp % npd) // 128)[sidx]
    sslot = (dstp % 128)[sidx]
    ssrc = pos[src][sidx]
    kk = sblk * cpb + rank // 128
    pp = rank % 128

    SRC = np.zeros((NDEV, 128, nchunk), np.int32)       # global table row of src
    SLOT = np.full((NDEV, 128, nchunk), 255.0, np.float32)
    SRC[sdev, pp, kk] = ssrc
    SLOT[sdev, pp, kk] = sslot

    # wrapped int16 index arrays for dma_gather, per group of GC chunks
    ng = nchunk // GC
    wcols = GC * 128 // 16
    srcw = np.zeros((NDEV, 128, wcols * ng), np.int16)
    for dv in range(NDEV):
        for g in range(ng):
            # edge i in group = c*128 + p, c in [0,GC)
            flat_s = SRC[dv][:, g * GC:(g + 1) * GC].T.reshape(-1)  # [GC*128] c-major
            srcw[dv][:, g * wcols:(g + 1) * wcols] = _wrap16(flat_s)

    # --- x permuted / padded / transposed ---
    xp = np.zeros((NDEV, npd, D), np.float32)
    xp[pos // npd, pos % npd] = x
    xpT = np.ascontiguousarray(xp.transpose(0, 2, 1))

    # --- weights: per-layer rotation folds e_src into hw'[:, 0] ---
    # B'_l = H_l @ diag(||a_src_l||, 1, ...) with Householder H_l e1 =
    # a_src_l/||a_src_l||, so (h @ W_l @ B'_l)[:, 0] = h @ W_l @ a_src_l =
    # e_src and the rest is an orthogonal rotation of hw. The table row is
    # then just 256 bf16 (512B): no separate e_src, no ones column. B'^{-1}
    # folds into layer l+1's weights host-side; layer 5's inverse is applied
    # on-device in the final matmul (binv5).
    W12 = np.ascontiguousarray(W1 @ W2)
    wfull = np.zeros((6, 257, 258), np.float32)
    Binv_prev = None
    for l in range(6):
        u = a_src[l].astype(np.float64)
        nu = float(np.linalg.norm(u))
        e1 = np.zeros(256, np.float64)
        e1[0] = 1.0
        v = u / nu - e1
        vv = float(v @ v)
        H = np.eye(256) - (2.0 / vv) * np.outer(v, v) if vv > 1e-12 else np.eye(256)
        Bp = H.copy()
        Bp[:, 0] *= nu
        Binv = H.copy()
        Binv[0, :] /= nu
        wext = np.concatenate(
            [Ws[l].astype(np.float64) @ Bp,
             (Ws[l] @ a_dst[l]).astype(np.float64)[:, None],
             np.zeros((256, 1))], axis=1
        )  # zero pad col: fp32r matmul needs even free width
        # layer 0 consumes x directly: fold the front MLP (W1 @ W2) in
        if l == 0:
            wfull[l, :256] = (W12.astype(np.float64) @ wext).astype(np.float32)
        else:
            wfull[l, :256] = (Binv_prev @ wext).astype(np.float32)
            wfull[l, 256] = (bias[l - 1].astype(np.float64) @ wext).astype(np.float32)
        Binv_prev = Binv
    W3s = np.ascontiguousarray(W3[:256] + W3[256:])
    binv5 = np.ascontiguousarray(
        Binv_prev.reshape(2, 128, 2, 128).transpose(1, 0, 2, 3)
    ).astype(np.float32)
    b5c = np.ascontiguousarray(bias[5].reshape(2, 128).T).astype(np.float32)
    iotaf = np.tile(np.arange(128, dtype=np.float32)[None, :], (128, 1))
    # static one-hot transpose per chunk: ptT[slot, k*128+e] = (slot(e,k) == slot)
    from concourse import mybir as _mb
    f8 = _mb.dt.np(_mb.dt.float8e4)
    PT8 = np.zeros((NDEV, 128, nchunk * 128), f8)
    for dv in range(NDEV):
        S = SLOT[dv].astype(np.int32)          # [128 e, nchunk k]
        e_i, k_i = np.nonzero(S < 128)
        PT8[dv][S[e_i, k_i], k_i * 128 + e_i] = 1.0

    in_maps = []
    for dv in range(NDEV):
        in_maps.append(
            {
                "xT": np.ascontiguousarray(xpT[dv]),
                "srcw": np.ascontiguousarray(srcw[dv]),
                "slotf": np.ascontiguousarray(SLOT[dv]),
                "wfull": wfull,
                "w3s": W3s,
                "binv5": binv5,
                "b5c": b5c,
                "iotaf": iotaf,
                "pt8": PT8[dv],
            }
        )
    return in_maps, pos, cpb, nchunk


# ---------------- bass program ----------------
def build(cfg, cpb, nchunk, knobs=None):
    kn = {"swdge": 4, "gpbufs": 8, "gsplit": 4}
    if knobs:
        kn.update(knobs)
    import concourse.bass as bass
    import concourse.bacc as bacc
    import concourse.tile as tile
    from concourse import mybir
    from concourse.masks import make_identity

    f32 = mybir.dt.float32
    f32r = mybir.dt.float32r
    bf16 = mybir.dt.bfloat16
    i16 = mybir.dt.int16
    AF = mybir.ActivationFunctionType
    OP = mybir.AluOpType
    npd, bpd = cfg.npd, cfg.bpd
    ng = nchunk // GC

    nc = bacc.Bacc(
        "TRN2",
        target_bir_lowering=False,
        debug=False,
        enable_asserts=False,
        num_devices=NDEV,
        num_swdge_queues=kn["swdge"],
    )
    xT = nc.dram_tensor("xT", [256, npd], f32, kind="ExternalInput").ap()
    wcols = GC * 128 // 16
    srcw = nc.dram_tensor("srcw", [128, wcols * ng], i16, kind="ExternalInput").ap()
    slotf = nc.dram_tensor("slotf", [128, nchunk], f32, kind="ExternalInput").ap()
    pt8 = nc.dram_tensor("pt8", [128, nchunk * 128], mybir.dt.float8e4,
                         kind="ExternalInput").ap()
    wfull = nc.dram_tensor("wfull", [6, 257, 258], f32, kind="ExternalInput").ap()
    w3s = nc.dram_tensor("w3s", [256, 256], f32, kind="ExternalInput").ap()
    binv5 = nc.dram_tensor("binv5", [128, 2, 2, 128], f32, kind="ExternalInput").ap()
    b5c = nc.dram_tensor("b5c", [128, 2], f32, kind="ExternalInput").ap()
    iotaf = nc.dram_tensor("iotaf", [128, 128], f32, kind="ExternalInput").ap()
    out = nc.dram_tensor("out", [npd, 256], f32, kind="ExternalOutput").ap()

    with tile.TileContext(nc) as tc:
        with (
            tc.tile_pool(name="cp", bufs=1) as cp,
            tc.tile_pool(name="sb", bufs=2) as sb,
            tc.tile_pool(name="gp", bufs=kn["gpbufs"]) as gp,
            tc.tile_pool(name="psA", bufs=3, space="PSUM") as psA,
            tc.tile_pool(name="psB", bufs=2, space="PSUM") as psB,
            tc.tile_pool(name="dp", bufs=1, space="DRAM") as dp,
        ):
            # ---- constants ----
            ident = cp.tile([128, 128], f32)
            make_identity(nc, ident[:])
            iota_f = cp.tile([128, 128], f32)
            nc.sync.dma_start(iota_f[:], iotaf)
            iota_b = cp.tile([128, 128], bf16)
            nc.vector.tensor_copy(iota_b[:], iota_f[:])
            # layer-invariant edge indexing data, resident in SBUF
            pt8_sb = cp.tile([128, nchunk * 128], mybir.dt.float8e4)
            nc.sync.dma_start(pt8_sb[:], pt8)
            srcw_all = cp.tile([128, wcols * ng], i16)
            nc.sync.dma_start(srcw_all[:], srcw)
            slot_all = cp.tile([128, nchunk], f32)
            nc.sync.dma_start(slot_all[:], slotf)
            ones1f = cp.tile([1, 128], f32)
            nc.gpsimd.memset(ones1f[:], 1.0)
            ones1 = cp.tile([1, 128], f32r)
            nc.vector.tensor_copy(ones1[:], ones1f[:])
            onecol = cp.tile([128, 1], bf16)
            nc.gpsimd.memset(onecol[:], 1.0)
            b5sb = cp.tile([128, 2], f32)
            nc.sync.dma_start(b5sb[:], b5c)
            # stage fp32 loads through a small buffer, DVE-cast to f32r
            wextsb = cp.tile([128, 6, 2, 258], f32r)
            bwsb = cp.tile([1, 6, 258], f32r)
            w3ssb = cp.tile([128, 2, 256], f32r)
            binv5sb = cp.tile([128, 2, 2, 128], f32r)

            def _stage(dst_ap, src_ap, shape):
                st = sb.tile(list(shape), f32, tag="wstage", bufs=2, name="wstage")
                nc.sync.dma_start(st[:], src_ap)
                nc.vector.tensor_copy(dst_ap, st[:])

            for l in range(6):
                _stage(
                    wextsb[:, l],
                    wfull[l, 0:256, :].rearrange("(a p) c -> p a c", p=128),
                    [128, 2, 258],
                )
            _stage(bwsb[:], wfull[:, 256:257, :].rearrange("l o c -> o l c"),
                   [1, 6, 258])
            _stage(w3ssb[:], w3s.rearrange("(a p) m -> p a m", p=128), [128, 2, 256])
            _stage(binv5sb[:], binv5, [128, 2, 2, 128])

            # ---- DRAM comm buffers (per layer: Shared tensors allow one writer) ----
            tbl_owns = [dp.tile([npd, RW], bf16, name=f"tbl_own{i}") for i in range(6)]
            tbl_fulls = [
                dp.tile([NDEV * npd, RW], bf16, addr_space="Shared", name=f"tbl_full{i}")
                for i in range(6)
            ]

            # ---- layer-0 hT is just xT (W1@W2 folded into wfull[0]) ----
            hT = sb.tile([128, 2, npd], f32r, tag="hT")
            xr = xT.rearrange("(a p) n -> p a n", p=128)
            xstep = min(512, npd)
            for n0 in range(0, npd, xstep):
                _stage(hT[:, :, n0:n0 + xstep], xr[:, :, n0:n0 + xstep],
                       [128, 2, xstep])

            for l in range(6):
                tbl_own, tbl_full = tbl_owns[l], tbl_fulls[l]
                # ---- own-node hw' + table build ----
                tbl_sb = sb.tile([128, bpd, RW], bf16, tag="tbl")
                edst_all = sb.tile([128, bpd], bf16, tag="edst")
                for b in range(bpd):
                    pshw = psB.tile([128, 258], f32, tag="pshw")
                    for ki in range(2):
                        nc.tensor.matmul(
                            pshw[:],
                            lhsT=hT[:, ki, b * 128 : (b + 1) * 128],
                            rhs=wextsb[:, l, ki, :],
                            start=(ki == 0),
                            stop=False,
                        )
                    nc.tensor.matmul(
                        pshw[:],
                        lhsT=ones1[:],
                        rhs=bwsb[:, l, :],
                        start=False,
                        stop=True,
                    )
                    nc.vector.tensor_copy(tbl_sb[:, b, 0:256], pshw[:, 0:256])
                    nc.vector.tensor_copy(edst_all[:, b : b + 1], pshw[:, 256:257])
                nc.sync.dma_start(
                    tbl_own[:].rearrange("(b p) c -> p b c", p=128), tbl_sb[:]
                )
                if AG_MODE == "tiny":
                    # timing probe: near-zero-byte collective, same structure
                    nc.gpsimd.collective_compute(
                        "AllGather",
                        mybir.AluOpType.bypass,
                        replica_groups=[list(range(NDEV))],
                        ins=[tbl_own[0:16]],
                        outs=[tbl_full[0:128]],
                    )
                else:
                    nc.gpsimd.collective_compute(
                        "AllGather",
                        mybir.AluOpType.bypass,
                        replica_groups=[list(range(NDEV))],
                        ins=[tbl_own[:]],
                        outs=[tbl_full[:]],
                    )

                # ---- e_dst per edge for ALL groups (overlaps the AllGather:
                # depends only on edst_all + the static one-hot transpose) ----
                EB_all = sb.tile([128, nchunk], f32, tag="eball")
                for g in range(ng):
                    pseb = psB.tile([128, GC], f32, tag="pseb", bufs=2)
                    for c in range(GC):
                        k = g * GC + c
                        b = k // cpb
                        nc.tensor.matmul(
                            pseb[:, c : c + 1],
                            lhsT=pt8_sb[:, k * 128:(k + 1) * 128],
                            rhs=edst_all[:, b : b + 1],
                            start=True,
                            stop=True,
                        )
                    nc.vector.tensor_copy(EB_all[:, g * GC:(g + 1) * GC], pseb[:])

                # ---- gather / scatter ----
                hT_next = sb.tile([128, 2, npd], f32r, tag="hT")
                pss = None
                for g in range(ng):
                    # --- group prologue: src gather + e_dst broadcast + ee ---
                    G = gp.tile([128, GC, RW], bf16, tag="G")
                    if SKIP != "gather":
                        gs = kn.get("gsplit", 1)
                        cs = GC // gs          # chunks per split
                        ws = cs * 128 // 16    # wrapped idx cols per split
                        for s in range(gs):
                            nc.gpsimd.dma_gather(
                                out_ap=G[:, s * cs:(s + 1) * cs, :],
                                in_ap=tbl_full[:],
                                idxs_ap=srcw_all[:, g * wcols + s * ws:
                                                  g * wcols + (s + 1) * ws],
                                num_idxs=cs * 128, num_idxs_reg=cs * 128,
                                elem_size=RW,
                                queue_num=(g * gs + s) % kn["swdge"],
                            )
                    X = gp.tile([128, 2 * GC], f32, tag="X")
                    nc.vector.tensor_tensor(
                        X[:, 0:GC], G[:, :, 0],
                        EB_all[:, g * GC:(g + 1) * GC], op=OP.add
                    )
                    nc.vector.tensor_scalar(
                        out=X[:, GC:2 * GC], in0=X[:, 0:GC], scalar1=NEG,
                        scalar2=None, op0=OP.mult,
                    )
                    E1 = gp.tile([128, 2 * GC], f32, tag="E1")
                    nc.scalar.activation(E1[:], X[:], AF.Exp)
                    EE = gp.tile([128, GC], f32, tag="EE")
                    nc.vector.tensor_tensor(
                        EE[:], E1[:, 0:GC], E1[:, GC:2 * GC], op=OP.max
                    )
                    # --- scatter pass ---
                    for c in range(GC):
                        k = g * GC + c
                        b, cc = divmod(k, cpb)
                        if cc == 0:
                            pss = psA.tile([128, 257], f32, tag="pss")
                        lt = gp.tile([128, 128], bf16, tag="lt")
                        nc.vector.tensor_scalar(
                            out=lt[:],
                            in0=iota_b[:],
                            scalar1=slot_all[:, k : k + 1],
                            scalar2=EE[:, c : c + 1],
                            op0=OP.is_equal,
                            op1=OP.mult,
                        )
                        nc.tensor.matmul(
                            pss[:, 0:256],
                            lhsT=lt[:],
                            rhs=G[:, c, :],
                            start=(cc == 0),
                            stop=(cc == cpb - 1),
                        )
                        # denominator column: same stationary lt, ones rhs.
                        # start=False always: the message matmul's start=True
                        # already cleared the whole bank's has_written bits
                        # (a second start here would re-clear them and drop
                        # chunk 0's messages); col 256's bit is clear, so the
                        # first write overwrites, later ones accumulate.
                        nc.tensor.matmul(
                            pss[:, 256:257],
                            lhsT=lt[:],
                            rhs=onecol[:],
                            start=False,
                            stop=(cc == cpb - 1),
                        )
                        if cc != cpb - 1:
                            continue
                        # ---- block epilogue: normalize ----
                        den = sb.tile([128, 1], f32, tag="den")
                        nc.vector.tensor_scalar(
                            out=den[:], in0=pss[:, 256:257], scalar1=1e-30,
                            scalar2=None, op0=OP.add,
                        )
                        rec = sb.tile([128, 1], f32, tag="rec")
                        nc.vector.reciprocal(rec[:], den[:])
                        onrm = sb.tile([128, 256], f32, tag="onrm")
                        nc.scalar.activation(
                            onrm[:], pss[:, 0:256], AF.Copy, scale=rec[:]
                        )
                        srct_t = onrm
                        pst = psB.tile([128, 256], f32, tag="pst", bufs=1)
                        for hh in range(2):
                            nc.tensor.transpose(
                                out=pst[:, hh * 128 : (hh + 1) * 128],
                                in_=srct_t[:, hh * 128 : (hh + 1) * 128],
                                identity=ident[:],
                            )
                            nc.vector.tensor_copy(
                                hT_next[:, hh, b * 128 : (b + 1) * 128],
                                pst[:, hh * 128 : (hh + 1) * 128],
                            )
                hT = hT_next

            # ---- final: h6 = n'5 @ B5^-1 + b5; out = relu(h6) @ (W3_top+W3_bot)
            # batch 4 node blocks per output DMA: fewer HWDGE dispatches
            OB = 4
            for b0 in range(0, bpd, OB):
                nb = min(OB, bpd - b0)
                oo = sb.tile([128, OB, 256], f32, tag="oo")
                for bi in range(nb):
                    b = b0 + bi
                    psf2 = psB.tile([128, 256], f32, tag="pst", bufs=1)
                    for dj in range(2):
                        for ki in range(2):
                            nc.tensor.matmul(
                                psf2[:, dj * 128 : (dj + 1) * 128],
                                lhsT=binv5sb[:, ki, dj],
                                rhs=hT[:, ki, b * 128 : (b + 1) * 128],
                                start=(ki == 0),
                                stop=(ki == 1),
                            )
                    zT = sb.tile([128, 2, 128], f32r, tag="zT")
                    for hh in range(2):
                        nc.vector.tensor_scalar(
                            out=zT[:, hh],
                            in0=psf2[:, hh * 128 : (hh + 1) * 128],
                            scalar1=b5sb[:, hh : hh + 1],
                            scalar2=0.0,
                            op0=OP.add,
                            op1=OP.max,
                        )
                    psf = psB.tile([128, 256], f32, tag="pshw")
                    for ki in range(2):
                        nc.tensor.matmul(
                            psf[:],
                            lhsT=zT[:, ki],
                            rhs=w3ssb[:, ki, :],
                            start=(ki == 0),
                            stop=(ki == 1),
                        )
                    nc.vector.tensor_copy(oo[:, bi, :], psf[:])
                nc.sync.dma_start(
                    out[b0 * 128 : (b0 + nb) * 128, :].rearrange(
                        "(b p) c -> p b c", p=128
                    ),
                    oo[:, 0:nb, :],
                )

    nc.compile()
    return nc


# ---------------- entry point ----------------
def kernel(**inputs):
    cfg = FULL
    in_maps, pos, cpb, nchunk = prep(inputs, cfg)
    nc = build(cfg, cpb, nchunk)
    from concourse import bass_utils

    res = bass_utils.run_bass_kernel_spmd(nc, in_maps, core_ids=list(range(NDEV)))
    outs = [res.results[dv]["out"] for dv in range(NDEV)]
    full = np.zeros((cfg.n, 256), np.float32)
    full[:] = np.stack(outs).reshape(NDEV * cfg.npd, 256)[pos]
    return full

